# revision 30
# baseline (speedup 1.0000x reference)
"""Trainium2 Bass kernel for fused attention block (QKV+gate proj, q/k RMS-norm,
RoPE, causal GQA attention, sigmoid gating, o_proj).

Sharding: 8 cores = 2 batches x 4 head-groups (tensor-parallel over heads,
data-parallel over batch). Each core computes a partial [T, D] output from its
4 q-heads / 1 kv-head; host sums the 4 partials per batch.

Self-contained: hardcodes all shapes; reads nothing from /root/problem.
"""

import os
import numpy as np
import ml_dtypes

import concourse.bass as bass
import concourse.bacc as bacc
import concourse.mybir as mybir
import concourse.tile as tile
from concourse.bass import ts, ds
from concourse.bass_utils import run_bass_kernel_spmd

# ---- problem constants ----
B, T, D = 2, 2048, 2048
NH, NKV, HD = 16, 4, 128
NQ = NH // NKV          # q heads per core
DH = NQ * HD            # 512 (attn feature rows per core)
EPS = 1e-6
SCALE = HD ** -0.5
TB = 512                # moving free-dim block
NTB = T // TB           # 4
NKT = D // 128          # 16 contraction tiles
NTT = T // 128          # 16 t(row)-tiles

F32 = mybir.dt.float32
BF16 = mybir.dt.bfloat16
F32R = mybir.dt.float32r
AF = mybir.ActivationFunctionType
NPBF16 = ml_dtypes.bfloat16

# matmul storage dtype: "bf16" or "f32r"
MM_MODE = os.environ.get("KERNEL_MM_MODE", "bf16")
MMDT = BF16 if MM_MODE == "bf16" else F32
NPMM = NPBF16 if MM_MODE == "bf16" else np.float32
WARM = int(os.environ.get("KERNEL_WARM", "56"))


def _mm(nc, out, lhsT, rhs, **kw):
    """matmul that goes through float32r when MM_MODE=f32r."""
    if MM_MODE == "f32r":
        lhsT = lhsT.bitcast(F32R)
        rhs = rhs.bitcast(F32R)
    nc.tensor.matmul(out, lhsT, rhs, **kw)


def _emit(tc, io):
    nc = tc.nc
    with (
        tc.tile_pool(name="consts", bufs=1) as cpool,
        tc.tile_pool(name="persist", bufs=1) as ppool,
        tc.tile_pool(name="xt", bufs=2) as xpool,
        tc.tile_pool(name="workB", bufs=2) as wb,
        tc.tile_pool(name="workBq", bufs=5) as wbq,
        tc.tile_pool(name="rows", bufs=2) as rows,
        tc.tile_pool(name="probs", bufs=6) as prp,
        tc.tile_pool(name="workC", bufs=2) as wc,
        tc.tile_pool(name="outp", bufs=4) as op,
        tc.tile_pool(name="ps_sc", bufs=3, space="PSUM") as ps_sc,
        tc.tile_pool(name="ps_acc", bufs=2, space="PSUM") as ps_acc,
        tc.tile_pool(name="ps_misc", bufs=2, space="PSUM") as ps_misc,
        tc.tile_pool(name="ps_den", bufs=1, space="PSUM") as ps_den,
    ):
        # ---------- PE warmup: keep PE continuously busy from t=0 so the HAM
        # clock gate un-throttles (~3.4us sustained) before real work, and PE
        # isn't idle while the first input DMAs land.
        warm_sb = cpool.tile([128, 128], MMDT, name="warm_sb")
        nc.gpsimd.memset(warm_sb[:], 0.0)
        warm_ps = ps_misc.tile([128, 128], F32, name="warm_ps", tag="misc")
        for _ in range(WARM):
            _mm(nc, warm_ps, warm_sb[:], warm_sb[:], start=True, stop=True)

        from concourse.masks import make_identity
        ident_sb = cpool.tile([128, 128], MMDT, name="ident_sb")
        make_identity(nc, ident_sb[:])

        # ---------- input DMAs. Three parallel HWDGE queues (sync, scalar,
        # vector), big contiguous ops (DRAM layouts are pre-blocked on host so
        # each op moves 4-16KB contiguous per partition).
        # sync: xt0 (4 quarter ops so k/q chains can start before the whole
        #       block lands), wq heads 1-3; later xt blocks + half the outs.
        # scalar: wk, wv, wg, wo.
        # vector: small tables + cos/sin + wq head 0.
        # The scalar queue gets ONLY the few critical-path early loads: each
        # DMA_DIRECT2D costs ~600ns of the issuing engine's time, and the
        # scalar engine must be free for the Squares that recycle the PSUM
        # accumulator ring once the chains start. Everything else goes on
        # sync (idle) and gpsimd (SWDGE, idle at start).
        wk_sb = cpool.tile([128, NKT, HD], MMDT, name="wk_sb")
        nc.scalar.dma_start(wk_sb[:], io["wk"][:, :, :])
        wq_sb = cpool.tile([128, NQ, NKT, HD], MMDT, name="wq_sb")
        nc.scalar.dma_start(wq_sb[:, 0, :, :], io["wq"][0, :, :, :])
        # xt0 in eighths alternating across both queues: the k-chain consumes
        # 4 matmuls per 2-kt chunk, so the PE is fed at the DMA arrival rate
        # without a single queue serializing the whole 2MB block.
        xt0 = xpool.tile([128, NKT, TB], MMDT, name="xt0", tag="xt")
        for e in range(8):
            eng = nc.sync if e % 2 == 0 else nc.scalar
            eng.dma_start(xt0[:, ts(e, 2), :], io["xb"][0, :, ts(e, 2), :])
        for h in range(1, NQ):
            nc.sync.dma_start(wq_sb[:, h, :, :], io["wq"][h, :, :, :])
        onesc_sb = cpool.tile([128, 1], MMDT, name="onesc_sb")
        nc.gpsimd.dma_start(onesc_sb[:], io["ones_col"][:, :])
        qw_sb = cpool.tile([128, 1], F32, name="qw_sb")
        nc.gpsimd.dma_start(qw_sb[:], io["qw_col"][:, :])
        kw_sb = cpool.tile([128, 1], F32, name="kw_sb")
        nc.gpsimd.dma_start(kw_sb[:], io["kw_col"][:, :])
        perm_sb = cpool.tile([128, HD], MMDT, name="perm_sb")
        nc.gpsimd.dma_start(perm_sb[:], io["perm"][:, :])
        tri_sb = cpool.tile([128, 128], MMDT, name="tri_sb")
        nc.gpsimd.dma_start(tri_sb[:], io["tri"][:, :])
        cos_sb = cpool.tile([128, T], MMDT, name="cos_sb")
        nc.gpsimd.dma_start(cos_sb[:], io["cosT"][:, :])
        sin_sb = cpool.tile([128, T], MMDT, name="sin_sb")
        nc.gpsimd.dma_start(sin_sb[:], io["sinT"][:, :])
        wv_sb = cpool.tile([128, NKT, HD], MMDT, name="wv_sb")
        nc.sync.dma_start(wv_sb[:], io["wv"][:, :, :])
        wg_sb = cpool.tile([128, NQ, NKT, HD], MMDT, name="wg_sb")
        for h in range(NQ):
            nc.sync.dma_start(wg_sb[:, h, :, :], io["wg"][h, :, :, :])
        eps_sb = cpool.tile([128, 1], F32, name="eps_sb")
        nc.gpsimd.memset(eps_sb[:], EPS)
        wo_sb = cpool.tile([128, NQ, D], MMDT, name="wo_sb")
        nc.sync.dma_start(wo_sb[:], io["wo"][:, :, :])

        # ---------- persistent activations ----------
        qrope = ppool.tile([128, NQ, T], MMDT, name="qrope")
        krope = ppool.tile([128, T], MMDT, name="krope")
        # sg holds tanh(gate/2); sigmoid(g) = 0.5*(1+tanh(g/2)) is finished
        # on DVE at use time (tanh shares the exp ACT table set).
        sg = ppool.tile([128, NQ, T], MMDT, name="sg")
        v_sb = ppool.tile([128, NTT, HD], MMDT, name="v_sb")
        attnT_t = [ppool.tile([128, NQ, TB], MMDT, name=f"attnT{i}")
                   for i in range(NTB)]

        osb4_box = {}
        pso_box = {}

        def emit_oproj_mm(src_tb, idx, hh):
            """One accumulation matmul of one o_proj tile (finest filler
            granularity); hh==0 allocates the PSUM tile, hh==NQ-1 finishes
            with the copy + (per half-row) output DMA."""
            tt, nb = divmod(idx, 4)
            ti = src_tb * 4 + tt
            if hh == 0:
                pso_box[ti, nb] = ps_misc.tile([128, TB], F32,
                                               name=f"pso_{ti}_{nb}", tag="misc")
            pso = pso_box[ti, nb]
            _mm(nc, pso, attnT_t[src_tb][:, hh, ts(tt, 128)],
                wo_sb[:, hh, ts(nb, TB)],
                start=(hh == 0), stop=(hh == NQ - 1))
            if hh == NQ - 1:
                _oproj_finish(src_tb, idx, pso)

        def _oproj_finish(src_tb, idx, pso):
            tt, nb = divmod(idx, 4)
            ti = src_tb * 4 + tt
            half, sub = divmod(nb, 2)
            if sub == 0:
                osb4_box[ti, half] = op.tile([128, 2, TB], MMDT,
                                             name=f"osb2_{ti}_{half}",
                                             tag="osb2", bufs=2)
            osb2 = osb4_box[ti, half]
            if idx % 2 == 0:
                nc.scalar.copy(osb2[:, sub, :], pso[:])
            else:
                nc.vector.tensor_copy(osb2[:, sub, :], pso[:])
            if sub == 1:
                eng = nc.gpsimd if (src_tb == NTB - 1 and half == 1) else nc.sync
                eng.dma_start(io["out2"][ti, half, :, :],
                              osb2[:].rearrange("p a b -> p (a b)"))

        def emit_oproj_idx(src_tb, idx):
            """One of the 16 o_proj tiles for query-block src_tb. The 4 nb
            tiles of one ti row share an osb4 tile and go out as a single
            512KB DMA (4KB contiguous per partition in the blocked out2
            layout) instead of 4x 128KB with 1KB runs."""
            for hh in range(NQ):
                emit_oproj_mm(src_tb, idx, hh)

        def emit_oproj_group(src_tb, g):
            for idx in range(4 * g, 4 * g + 4):
                emit_oproj_idx(src_tb, idx)

        import contextlib
        reps = _REPS[0]
        loop_ctx = tc.For_i(0, reps, 1) if reps > 1 else contextlib.nullcontext()
        with loop_ctx:
         def make_phaseB(tb, defer_gates=False):
            """Emit the xt DMA for block tb now; return the projection
            step-closures (accum blocks with their staggered norm/rope tails)
            for later interleaved emission. With defer_gates, the 4 gate
            chains are returned separately (emitted per-head inside phase C)."""
            tsl = ds(tb * TB, TB)
            if tb == 0:
                xt = xt0
            else:
                xt = xpool.tile([128, NKT, TB], MMDT, name="xt", tag="xt")
                nc.sync.dma_start(xt[:, 0:8, :], io["xb"][tb, :, 0:8, :])
                nc.sync.dma_start(xt[:, 8:16, :], io["xb"][tb, :, 8:16, :])

            # k first: its weight DMA lands earliest, so PE starts sooner
            qk_specs = [("k", 0)] + [("q", h) for h in range(NQ)]
            tails = {}
            vt_store = {}

            def accum_qk(i, tb=tb, xt=xt, tails=tails, qk_specs=qk_specs):
                kind, h = qk_specs[i]
                ps = ps_acc.tile([128, TB], F32, name=f"psqk_{tb}_{i}", tag="acc")
                for kt in range(NKT):
                    lhsT = wq_sb[:, h, kt, :] if kind == "q" else wk_sb[:, kt, :]
                    _mm(nc, ps, lhsT, xt[:, kt, :], start=(kt == 0), stop=(kt == NKT - 1))
                sq = wbq.tile([128, TB], MMDT, name=f"sq_{tb}_{i}", tag="sq")
                nc.scalar.activation(sq[:], ps[:], AF.Square)
                qsb = wbq.tile([128, TB], MMDT, name=f"qsb_{tb}_{i}", tag="qsb")
                w_col = qw_sb if kind == "q" else kw_sb
                nc.vector.tensor_scalar_mul(qsb[:], ps[:], w_col[:, 0:1])
                tails[i] = {"sq": sq, "qsb": qsb, "kind": kind, "h": h}

            def tail_var(i, tb=tb, tails=tails):
                st = tails[i]
                vps = ps_misc.tile([1, TB], F32, name=f"var_{tb}_{i}", tag="misc")
                _mm(nc, vps, onesc_sb[:, :], st["sq"][:, :], start=True, stop=True)
                # ones_col is 2.0 (for the sigmoid-via-tanh trick), so vps is
                # 2*sum(q^2); the extra 2 is folded into the sqrt scale.
                # The Sqrt lives in a different ACT table set than Exp; all 5
                # tail_vars of a block are emitted back-to-back so the table
                # reload happens twice per block instead of per chain.
                srow = rows.tile([1, TB], F32, name=f"srow_{tb}_{i}", tag="srow")
                nc.scalar.activation(srow[:], vps[:], AF.Sqrt,
                                     bias=eps_sb[0:1, 0:1], scale=0.5 / HD)
                vrow = rows.tile([1, TB], F32, name=f"vrow_{tb}_{i}", tag="row")
                nc.vector.reciprocal_approx_fast(out=vrow[:], in_=srow[:])
                vrep = wb.tile([128, TB], F32, name=f"vrep_{tb}_{i}", tag="vrep",
                               bufs=3)
                nc.gpsimd.partition_broadcast(vrep[:], vrow[0:1, :])
                st["vrep"] = vrep

            def tail_rot(i, tb=tb, tsl=tsl, tails=tails):
                # RoPE on the UN-normalized qsb: the per-token rstd is a
                # column scalar, so it commutes with the rotation and is
                # applied afterwards in tail_fm. This removes the PE rot
                # matmul from the sqrt->recip->bcast serial chain.
                st = tails[i]
                rot = ps_misc.tile([128, TB], F32, name=f"rot_{tb}_{i}", tag="misc")
                _mm(nc, rot[:], perm_sb[:, :], st["qsb"][:, :], start=True, stop=True)
                rot_sb = wb.tile([128, TB], MMDT, name=f"rsb_{tb}_{i}", tag="rsb")
                nc.scalar.copy(rot_sb[:], rot[:])
                t1 = wb.tile([128, TB], MMDT, name=f"t1_{tb}_{i}", tag="t1")
                nc.vector.tensor_mul(t1[:], st["qsb"][:], cos_sb[:, tsl])
                t2 = wb.tile([128, TB], MMDT, name=f"t2_{tb}_{i}", tag="t2")
                nc.vector.tensor_mul(t2[:], rot_sb[:], sin_sb[:, tsl])
                qtmp = wb.tile([128, TB], MMDT, name=f"qtmp_{tb}_{i}", tag="qn",
                               bufs=4)
                nc.vector.tensor_add(qtmp[:], t1[:], t2[:])
                st["qtmp"] = qtmp

            def tail_fm(i, tb=tb, tsl=tsl, tails=tails):
                st = tails[i]
                dst = qrope[:, st["h"], tsl] if st["kind"] == "q" else krope[:, tsl]
                nc.vector.tensor_mul(dst, st["qtmp"][:], st["vrep"][:])

            def accum_gate(h, tb=tb, xt=xt, tsl=tsl):
                ps = ps_acc.tile([128, TB], F32, name=f"psg_{tb}_{h}", tag="acc")
                for kt in range(NKT):
                    _mm(nc, ps, wg_sb[:, h, kt, :], xt[:, kt, :],
                        start=(kt == 0), stop=(kt == NKT - 1))
                # sigmoid(g) = 0.5*(1+tanh(g/2)); tanh shares the ACT table
                # set with exp so this causes no table reloads. The 0.5 is
                # folded into ones_col=2.0 (den becomes 2*den -> rden halves).
                nc.scalar.activation(sg[:, h, tsl], ps[:], AF.Tanh, scale=0.5)

            def accum_vT(tb=tb, xt=xt, vt_box=None):
                ps = ps_acc.tile([128, TB], F32, name=f"psvT_{tb}", tag="acc")
                for kt in range(NKT):
                    _mm(nc, ps, wv_sb[:, kt, :], xt[:, kt, :],
                        start=(kt == 0), stop=(kt == NKT - 1))
                vt = wb.tile([128, TB], MMDT, name=f"vt_{tb}", tag="vt")
                nc.vector.tensor_copy(vt[:], ps[:])
                vt_store[tb] = vt

            def transpose_v(tt, tb=tb):
                ti = tb * 4 + tt
                ps = ps_misc.tile([128, HD], MMDT, name=f"psvt_{tb}_{tt}", tag="misc")
                nc.tensor.transpose(ps[:], vt_store[tb][:, ts(tt, 128)], ident_sb[:])
                nc.vector.tensor_copy(v_sb[:, ti, :], ps[:])

            blocks = ([lambda i=i: accum_qk(i) for i in range(5)]
                      + [accum_vT])
            if not defer_gates:
                blocks += [lambda h=h: accum_gate(h) for h in range(NQ)]
            # All 5 tail_vars are emitted in one step so their Sqrt ops are
            # consecutive on the ACT queue (2 table reloads per block, not
            # 10). The rot matmuls move to later steps so the gate chains
            # provide PE cover while the serial sqrt->recip->bcast->qn chain
            # completes (otherwise the PE stalls ~2.5us once per block).
            tail_sched = {}
            for i in range(4):
                tail_sched.setdefault(i + 2, []).append(lambda i=i: tail_rot(i))
            # k/q0 rstd chains start 2 steps early so qrope/krope are ready
            # well before the next C phase's first scores (the sqrt->recip->
            # bcast->fm chain has ~3us latency and was landing just-in-time)
            fns = {'v': tail_var, 'f': tail_fm, 'r': tail_rot}
            for key, order in ((4, [('v', 0), ('v', 1), ('f', 0)]),
                               (5, [('v', 2), ('f', 1)]),
                               (6, [('r', 4), ('v', 3), ('f', 2), ('v', 4),
                                    ('f', 3), ('f', 4)])):
                tail_sched.setdefault(key, []).extend(
                    (lambda k=k, i=i: fns[k](i)) for k, i in order)
            for tt in range(4):
                tail_sched.setdefault(7 + tt, []).append(lambda tt=tt: transpose_v(tt))

            def step(bi, blk):
                if blk is not None:
                    blk()
                for fn in tail_sched.get(bi + 1, ()):
                    fn()
            n_steps = max(len(blocks), max(tail_sched) if tail_sched else 0)
            steps = [lambda bi=bi: step(bi, blocks[bi] if bi < len(blocks) else None)
                     for bi in range(n_steps)]
            n = len(steps)
            groups = [steps[(n * h) // NQ:(n * (h + 1)) // NQ] for h in range(NQ)]
            if defer_gates:
                return groups, [lambda h=h: accum_gate(h) for h in range(NQ)]
            return groups

         def emit_phaseC(tb, fill_steps, pre_steps=None):
            """Attention for query block tb; after each head also emit the
            deferred o_proj group of tb-1 plus a slice of the next block's
            projection steps (PE work independent of the exp pipeline)."""
            tsl = ds(tb * TB, TB)
            nj = 4 * (tb + 1)
            for h in range(NQ):
                if pre_steps is not None and pre_steps[h] is not None:
                    pre_steps[h]()
                attn_ps = ps_acc.tile([128, TB], F32, name=f"attn_{tb}_{h}", tag="acc")
                den_ps = ps_den.tile([1, TB], F32, name=f"den_{tb}_{h}", tag="den")
                probs_t = [None] * nj

                def emit_scores(j, h=h, tb=tb, tsl=tsl, probs_t=probs_t):
                    o = j - 4 * tb
                    c0 = max(0, o) * 128          # first valid column in this tile
                    w = TB - c0
                    sp = ps_sc.tile([128, TB], F32, name=f"sc_{tb}_{h}_{j}", tag="sc")
                    _mm(nc, sp[:, c0:], krope[:, ts(j, 128)],
                        qrope[:, h, ds(tb * TB + c0, w)], start=True, stop=True)
                    pr = prp.tile([128, TB], MMDT, name=f"pr_{tb}_{h}_{j}", tag="pr")
                    nc.scalar.activation(pr[:, c0:], sp[:, c0:], AF.Exp, scale=SCALE)
                    if o >= 0:
                        nc.vector.tensor_mul(pr[:, c0:c0 + 128], pr[:, c0:c0 + 128],
                                             tri_sb[:, :])
                    probs_t[j] = pr

                # den: off-diagonal prob tiles are summed in groups of 4 on
                # DVE, one ones^T@sum matmul per group (saves 3 PE matmuls
                # per group); diagonal tiles keep per-j matmuls (partial
                # width, columns below c0 are unwritten).
                den_first = 3 if tb > 0 else 0

                def emit_av(j, h=h, tb=tb, nj=nj, attn_ps=attn_ps, den_ps=den_ps,
                            probs_t=probs_t, den_first=den_first):
                    o = j - 4 * tb
                    c0 = max(0, o) * 128
                    pr = probs_t[j]
                    _mm(nc, attn_ps[:, c0:], v_sb[:, j, :], pr[:, c0:],
                        start=(j == 0), stop=(j == nj - 1))
                    if o >= 0:
                        _mm(nc, den_ps[:, c0:], onesc_sb[:, :], pr[:, c0:],
                            start=(j == den_first), stop=(j == nj - 1))
                    elif j % 4 == 3:
                        p0, p1, p2, p3 = (probs_t[j - 3], probs_t[j - 2],
                                          probs_t[j - 1], probs_t[j])
                        s4 = wc.tile([128, TB], MMDT, name=f"s4_{tb}_{h}_{j}",
                                     tag="s4")
                        nc.vector.tensor_add(s4[:], p0[:], p1[:])
                        nc.vector.tensor_add(s4[:], s4[:], p2[:])
                        nc.vector.tensor_add(s4[:], s4[:], p3[:])
                        _mm(nc, den_ps[:, :], onesc_sb[:, :], s4[:, :],
                            start=(j == den_first), stop=False)

                # o_proj work of the previous query block is interleaved
                # into the j-loop at single-matmul granularity: the exp
                # pipeline produces one prob tile per ~600ns while PE only
                # has ~426ns of attn work per tile, so one extra independent
                # matmul per iteration keeps the PE from stalling on ACT.
                fillers = ([lambda idx=idx, hh=hh: emit_oproj_mm(tb - 1, idx, hh)
                            for idx in range(4 * h, 4 * h + 4)
                            for hh in range(NQ)]
                           if tb > 0 else [])
                fi = 0
                LOOK = 3
                for j in range(nj):
                    emit_scores(j)
                    if j >= LOOK:
                        emit_av(j - LOOK)
                    if j >= 2 and fi < len(fillers):
                        fillers[fi]()
                        fi += 1
                for j in range(max(0, nj - LOOK), nj):
                    emit_av(j)
                while fi < len(fillers):
                    fillers[fi]()
                    fi += 1

                # normalize + gate: attnT = attn/(2 den) * (1 + tanh(g/2))
                # (den here is 2*true_den because ones_col=2.0, so the 0.5 of
                # the sigmoid-from-tanh identity is already folded in)
                drow = rows.tile([1, TB], F32, name=f"drow_{tb}_{h}", tag="row")
                nc.vector.reciprocal_approx_fast(out=drow[:], in_=den_ps[:])
                rden = wc.tile([128, TB], F32, name=f"rden_{tb}_{h}", tag="rden")
                nc.gpsimd.partition_broadcast(rden[:], drow[0:1, :])
                g1 = wc.tile([128, TB], MMDT, name=f"g1_{tb}_{h}", tag="g1")
                nc.vector.tensor_mul(g1[:], attn_ps[:], rden[:])
                nc.vector.scalar_tensor_tensor(
                    out=attnT_t[tb][:, h, :], in0=sg[:, h, tsl], scalar=1.0,
                    in1=g1[:], op0=mybir.AluOpType.add, op1=mybir.AluOpType.mult)

                if fill_steps:
                    for s in fill_steps[h]:
                        s()

         # driver: B0 (gates deferred into C0), then C(tb) with B(tb+1)
         # groups interleaved
         b0_groups, b0_gates = make_phaseB(0, defer_gates=True)
         b0_steps = [st for grp in b0_groups for st in grp]
         for st in b0_steps[:6]:      # chains + vT + var batch
            st()
         b0_gates[0]()                # gate chain h0: PE cover for the
         for st in b0_steps[6:]:      # sqrt->qn chain feeding the rots
            st()
         nextB = make_phaseB(1)
         c0_pre = [None] + b0_gates[1:]
         for tb in range(NTB):
            emit_phaseC(tb, nextB, pre_steps=c0_pre if tb == 0 else None)
            nextB = make_phaseB(tb + 2) if tb + 2 < NTB else None

         # final o_proj for the last query block
         for g in range(4):
            emit_oproj_group(NTB - 1, g)


_CACHED = {}
_REPS = [1]


def _build(reps=None):
    if reps is None:
        reps = int(os.environ.get("KERNEL_REPS", "1"))
    if reps in _CACHED:
        return _CACHED[reps]
    _REPS[0] = reps
    nc = bacc.Bacc("TRN2", target_bir_lowering=False, debug=False, num_devices=8)
    io = {}
    def din(name, shape, dt):
        io[name] = nc.dram_tensor(name, shape, dt, kind="ExternalInput").ap()
    din("xb", [NTB, 128, NKT, TB], MMDT)
    din("wq", [NQ, 128, NKT, HD], MMDT)
    din("wg", [NQ, 128, NKT, HD], MMDT)
    din("wk", [128, NKT, HD], MMDT)
    din("wv", [128, NKT, HD], MMDT)
    din("wo", [128, NQ, D], MMDT)
    din("cosT", [HD, T], MMDT)
    din("sinT", [HD, T], MMDT)
    din("perm", [HD, HD], MMDT)
    din("qw_col", [HD, 1], F32)
    din("kw_col", [HD, 1], F32)
    din("tri", [128, 128], MMDT)
    din("ones_col", [128, 1], MMDT)
    io["out2"] = nc.dram_tensor("out2", [NTT, 2, 128, 2 * TB], MMDT,
                                kind="ExternalOutput").ap()

    with tile.TileContext(nc, num_cores=8) as tc:
        _emit(tc, io)
    nc.compile()
    _CACHED[reps] = nc
    return nc


def _prep_in_maps(inputs):
    hidden = np.asarray(inputs["hidden_BTD"], np.float32)
    cos = np.asarray(inputs["cos_BTK"], np.float32)
    sin = np.asarray(inputs["sin_BTK"], np.float32)
    w_q = np.asarray(inputs["w_q"], np.float32)
    w_k = np.asarray(inputs["w_k"], np.float32)
    w_v = np.asarray(inputs["w_v"], np.float32)
    w_o = np.asarray(inputs["w_o"], np.float32)
    qw = np.asarray(inputs["q_norm_w"], np.float32)
    kw = np.asarray(inputs["k_norm_w"], np.float32)

    wq4 = w_q.reshape(D, NH, 2 * HD)

    def cvt(x):
        return np.ascontiguousarray(np.asarray(x, np.float32).astype(NPMM))

    # upper-tri-inclusive [128,128]: tri[jl, cc] = 1 iff jl <= cc
    tri = np.triu(np.ones((128, 128), np.float32))

    perm = np.zeros((128, 128), np.float32)
    perm[np.arange(64), np.arange(64) + 64] = 1.0
    perm[np.arange(64, 128), np.arange(64, 128) - 64] = -1.0

    def blk_w(w):  # [D, C] -> [128, NKT, C] with d = kt*128 + p
        return w.reshape(NKT, 128, -1).transpose(1, 0, 2)

    in_maps = []
    for c in range(8):
        b, g = divmod(c, 4)
        heads = list(range(4 * g, 4 * g + 4))
        xT = hidden[b].T                                   # [D, T]
        # [tb][p][kt][t'] blocks, contiguous per block
        xb = xT.reshape(NKT, 128, NTB, TB).transpose(2, 1, 0, 3)
        wq_h = np.stack([blk_w(wq4[:, h, :HD]) for h in heads])   # [NQ,128,NKT,HD]
        wg_h = np.stack([blk_w(wq4[:, h, HD:]) for h in heads])
        wo_g = w_o[4 * g * HD:(4 * g + 4) * HD, :]         # [DH, D]
        m = {
            "xb": cvt(xb),
            "wq": cvt(wq_h),
            "wg": cvt(wg_h),
            "wk": cvt(blk_w(w_k[:, g * HD:(g + 1) * HD])),
            "wv": cvt(blk_w(w_v[:, g * HD:(g + 1) * HD])),
            "wo": cvt(wo_g.reshape(NQ, 128, D).transpose(1, 0, 2)),
            "cosT": cvt(cos[b].T),
            "sinT": cvt(sin[b].T),
            "perm": cvt(perm),
            "qw_col": np.ascontiguousarray(qw[:, None]),
            "kw_col": np.ascontiguousarray(kw[:, None]),
            "tri": cvt(tri),
            # 2.0 (not 1.0): folds the 0.5 of sigmoid(x)=0.5*(1+tanh(x/2))
            # into the softmax denominator; the var matmul's extra 2 is
            # compensated in the Sqrt scale.
            "ones_col": cvt(np.full((128, 1), 2.0, np.float32)),
        }
        in_maps.append(m)
    return in_maps


def run(inputs, **spmd_kwargs):
    """Build+run; returns (full_output [B,T,D] fp32, BassKernelResults)."""
    nc = _build()
    in_maps = _prep_in_maps(inputs)
    res = run_bass_kernel_spmd(nc, in_maps, core_ids=list(range(8)), **spmd_kwargs)
    out = np.zeros((B, T, D), np.float32)
    for c in range(8):
        o = np.asarray(res.results[c]["out2"], np.float32)  # [NTT,2,128,1024]
        out[c // 4] += o.transpose(0, 2, 1, 3).reshape(T, D)
    return out, res


def kernel(**inputs):
    out, _ = run(inputs)
    return out


# revision 31
# speedup vs baseline: 1.1678x; 1.1678x over previous
"""Trainium2 Bass kernel for fused attention block (QKV+gate proj, q/k RMS-norm,
RoPE, causal GQA attention, sigmoid gating, o_proj).

Sharding: 8 cores = 2 batches x 4 head-groups (tensor-parallel over heads,
data-parallel over batch). Each core computes a partial [T, D] output from its
4 q-heads / 1 kv-head; host sums the 4 partials per batch.

Self-contained: hardcodes all shapes; reads nothing from /root/problem.
"""

import os
import numpy as np
import ml_dtypes

import concourse.bass as bass
import concourse.bacc as bacc
import concourse.mybir as mybir
import concourse.tile as tile
from concourse.bass import ts, ds
from concourse.bass_utils import run_bass_kernel_spmd

# ---- problem constants ----
B, T, D = 2, 2048, 2048
NH, NKV, HD = 16, 4, 128
NQ = NH // NKV          # q heads per core
DH = NQ * HD            # 512 (attn feature rows per core)
EPS = 1e-6
SCALE = HD ** -0.5
TB = 512                # moving free-dim block
NTB = T // TB           # 4
NKT = D // 128          # 16 contraction tiles
NTT = T // 128          # 16 t(row)-tiles

F32 = mybir.dt.float32
BF16 = mybir.dt.bfloat16
F32R = mybir.dt.float32r
AF = mybir.ActivationFunctionType
NPBF16 = ml_dtypes.bfloat16

# matmul storage dtype: "bf16" or "f32r"
MM_MODE = os.environ.get("KERNEL_MM_MODE", "bf16")
MMDT = BF16 if MM_MODE == "bf16" else F32
NPMM = NPBF16 if MM_MODE == "bf16" else np.float32
WARM = int(os.environ.get("KERNEL_WARM", "56"))


def _mm(nc, out, lhsT, rhs, **kw):
    """matmul that goes through float32r when MM_MODE=f32r."""
    if MM_MODE == "f32r":
        lhsT = lhsT.bitcast(F32R)
        rhs = rhs.bitcast(F32R)
    nc.tensor.matmul(out, lhsT, rhs, **kw)


def _emit(tc, io):
    nc = tc.nc
    with (
        tc.tile_pool(name="consts", bufs=1) as cpool,
        tc.tile_pool(name="persist", bufs=1) as ppool,
        tc.tile_pool(name="xt", bufs=2) as xpool,
        tc.tile_pool(name="workB", bufs=2) as wb,
        tc.tile_pool(name="workBq", bufs=5) as wbq,
        tc.tile_pool(name="rows", bufs=2) as rows,
        tc.tile_pool(name="probs", bufs=6) as prp,
        tc.tile_pool(name="workC", bufs=2) as wc,
        tc.tile_pool(name="outp", bufs=4) as op,
        tc.tile_pool(name="ps_sc", bufs=3, space="PSUM") as ps_sc,
        tc.tile_pool(name="ps_acc", bufs=2, space="PSUM") as ps_acc,
        tc.tile_pool(name="ps_misc", bufs=2, space="PSUM") as ps_misc,
        tc.tile_pool(name="ps_den", bufs=1, space="PSUM") as ps_den,
    ):
        # ---------- PE warmup: keep PE continuously busy from t=0 so the HAM
        # clock gate un-throttles (~3.4us sustained) before real work, and PE
        # isn't idle while the first input DMAs land.
        warm_sb = cpool.tile([128, 128], MMDT, name="warm_sb")
        nc.gpsimd.memset(warm_sb[:], 0.0)
        warm_ps = ps_misc.tile([128, 128], F32, name="warm_ps", tag="misc")
        for _ in range(WARM):
            _mm(nc, warm_ps, warm_sb[:], warm_sb[:], start=True, stop=True)

        from concourse.masks import make_identity
        ident_sb = cpool.tile([128, 128], MMDT, name="ident_sb")
        make_identity(nc, ident_sb[:])

        # ---------- input DMAs. Three parallel HWDGE queues (sync, scalar,
        # vector), big contiguous ops (DRAM layouts are pre-blocked on host so
        # each op moves 4-16KB contiguous per partition).
        # sync: xt0 (4 quarter ops so k/q chains can start before the whole
        #       block lands), wq heads 1-3; later xt blocks + half the outs.
        # scalar: wk, wv, wg, wo.
        # vector: small tables + cos/sin + wq head 0.
        # The scalar queue gets ONLY the few critical-path early loads: each
        # DMA_DIRECT2D costs ~600ns of the issuing engine's time, and the
        # scalar engine must be free for the Squares that recycle the PSUM
        # accumulator ring once the chains start. Everything else goes on
        # sync (idle) and gpsimd (SWDGE, idle at start).
        wk_sb = cpool.tile([128, NKT, HD], MMDT, name="wk_sb")
        nc.scalar.dma_start(wk_sb[:], io["wk"][:, :, :])
        wq_sb = cpool.tile([128, NQ, NKT, HD], MMDT, name="wq_sb")
        nc.scalar.dma_start(wq_sb[:, 0, :, :], io["wq"][0, :, :, :])
        # xt0 in eighths alternating across both queues: the k-chain consumes
        # 4 matmuls per 2-kt chunk, so the PE is fed at the DMA arrival rate
        # without a single queue serializing the whole 2MB block.
        xt0 = xpool.tile([128, NKT, TB], MMDT, name="xt0", tag="xt")
        for e in range(8):
            eng = nc.sync if e % 2 == 0 else nc.scalar
            eng.dma_start(xt0[:, ts(e, 2), :], io["xb"][0, :, ts(e, 2), :])
        for h in range(1, NQ):
            nc.sync.dma_start(wq_sb[:, h, :, :], io["wq"][h, :, :, :])
        onesc_sb = cpool.tile([128, 1], MMDT, name="onesc_sb")
        nc.gpsimd.dma_start(onesc_sb[:], io["ones_col"][:, :])
        qw_sb = cpool.tile([128, 1], F32, name="qw_sb")
        nc.gpsimd.dma_start(qw_sb[:], io["qw_col"][:, :])
        kw_sb = cpool.tile([128, 1], F32, name="kw_sb")
        nc.gpsimd.dma_start(kw_sb[:], io["kw_col"][:, :])
        perm_sb = cpool.tile([128, HD], MMDT, name="perm_sb")
        nc.gpsimd.dma_start(perm_sb[:], io["perm"][:, :])
        tri_sb = cpool.tile([128, 128], MMDT, name="tri_sb")
        nc.gpsimd.dma_start(tri_sb[:], io["tri"][:, :])
        cos_sb = cpool.tile([128, T], MMDT, name="cos_sb")
        nc.gpsimd.dma_start(cos_sb[:], io["cosT"][:, :])
        sin_sb = cpool.tile([128, T], MMDT, name="sin_sb")
        nc.gpsimd.dma_start(sin_sb[:], io["sinT"][:, :])
        wv_sb = cpool.tile([128, NKT, HD], MMDT, name="wv_sb")
        nc.sync.dma_start(wv_sb[:], io["wv"][:, :, :])
        wg_sb = cpool.tile([128, NQ, NKT, HD], MMDT, name="wg_sb")
        for h in range(NQ):
            nc.sync.dma_start(wg_sb[:, h, :, :], io["wg"][h, :, :, :])
        eps_sb = cpool.tile([128, 1], F32, name="eps_sb")
        nc.gpsimd.memset(eps_sb[:], EPS)
        wo_sb = cpool.tile([128, NQ, D], MMDT, name="wo_sb")
        nc.sync.dma_start(wo_sb[:], io["wo"][:, :, :])

        # ---------- persistent activations ----------
        qrope = ppool.tile([128, NQ, T], MMDT, name="qrope")
        krope = ppool.tile([128, T], MMDT, name="krope")
        # sg holds tanh(gate/2); sigmoid(g) = 0.5*(1+tanh(g/2)) is finished
        # on DVE at use time (tanh shares the exp ACT table set).
        sg = ppool.tile([128, NQ, T], MMDT, name="sg")
        v_sb = ppool.tile([128, NTT, HD], MMDT, name="v_sb")
        attnT_t = [ppool.tile([128, NQ, TB], MMDT, name=f"attnT{i}")
                   for i in range(NTB)]

        osb4_box = {}
        pso_box = {}

        def emit_oproj_mm(src_tb, idx, hh):
            """One accumulation matmul of one o_proj tile (finest filler
            granularity); hh==0 allocates the PSUM tile, hh==NQ-1 finishes
            with the copy + (per half-row) output DMA."""
            tt, nb = divmod(idx, 4)
            ti = src_tb * 4 + tt
            if hh == 0:
                pso_box[ti, nb] = ps_misc.tile([128, TB], F32,
                                               name=f"pso_{ti}_{nb}", tag="misc")
            pso = pso_box[ti, nb]
            _mm(nc, pso, attnT_t[src_tb][:, hh, ts(tt, 128)],
                wo_sb[:, hh, ts(nb, TB)],
                start=(hh == 0), stop=(hh == NQ - 1))
            if hh == NQ - 1:
                _oproj_finish(src_tb, idx, pso)

        def _oproj_finish(src_tb, idx, pso):
            tt, nb = divmod(idx, 4)
            ti = src_tb * 4 + tt
            half, sub = divmod(nb, 2)
            if sub == 0:
                osb4_box[ti, half] = op.tile([128, 2, TB], MMDT,
                                             name=f"osb2_{ti}_{half}",
                                             tag="osb2", bufs=2)
            osb2 = osb4_box[ti, half]
            if idx % 2 == 0:
                nc.scalar.copy(osb2[:, sub, :], pso[:])
            else:
                nc.vector.tensor_copy(osb2[:, sub, :], pso[:])
            if sub == 1:
                eng = nc.gpsimd if (src_tb == NTB - 1 and half == 1) else nc.sync
                eng.dma_start(io["out2"][ti, half, :, :],
                              osb2[:].rearrange("p a b -> p (a b)"))

        def emit_oproj_idx(src_tb, idx):
            """One of the 16 o_proj tiles for query-block src_tb. The 4 nb
            tiles of one ti row share an osb4 tile and go out as a single
            512KB DMA (4KB contiguous per partition in the blocked out2
            layout) instead of 4x 128KB with 1KB runs."""
            for hh in range(NQ):
                emit_oproj_mm(src_tb, idx, hh)

        def emit_oproj_group(src_tb, g):
            for idx in range(4 * g, 4 * g + 4):
                emit_oproj_idx(src_tb, idx)

        import contextlib
        reps = _REPS[0]
        loop_ctx = tc.For_i(0, reps, 1) if reps > 1 else contextlib.nullcontext()
        with loop_ctx:
         def make_phaseB(tb, defer_gates=False):
            """Emit the xt DMA for block tb now; return the projection
            step-closures (accum blocks with their staggered norm/rope tails)
            for later interleaved emission. With defer_gates, the 4 gate
            chains are returned separately (emitted per-head inside phase C)."""
            tsl = ds(tb * TB, TB)
            if tb == 0:
                xt = xt0
            else:
                xt = xpool.tile([128, NKT, TB], MMDT, name="xt", tag="xt")
                nc.sync.dma_start(xt[:, 0:8, :], io["xb"][tb, :, 0:8, :])
                nc.sync.dma_start(xt[:, 8:16, :], io["xb"][tb, :, 8:16, :])

            # k first: its weight DMA lands earliest, so PE starts sooner
            qk_specs = [("k", 0)] + [("q", h) for h in range(NQ)]
            tails = {}
            vt_store = {}

            def accum_qk(i, tb=tb, xt=xt, tails=tails, qk_specs=qk_specs):
                kind, h = qk_specs[i]
                ps = ps_acc.tile([128, TB], F32, name=f"psqk_{tb}_{i}", tag="acc")
                for kt in range(NKT):
                    lhsT = wq_sb[:, h, kt, :] if kind == "q" else wk_sb[:, kt, :]
                    _mm(nc, ps, lhsT, xt[:, kt, :], start=(kt == 0), stop=(kt == NKT - 1))
                sq = wbq.tile([128, TB], MMDT, name=f"sq_{tb}_{i}", tag="sq")
                nc.scalar.activation(sq[:], ps[:], AF.Square)
                qsb = wbq.tile([128, TB], MMDT, name=f"qsb_{tb}_{i}", tag="qsb")
                w_col = qw_sb if kind == "q" else kw_sb
                nc.vector.tensor_scalar_mul(qsb[:], ps[:], w_col[:, 0:1])
                tails[i] = {"sq": sq, "qsb": qsb, "kind": kind, "h": h}

            def tail_var(i, tb=tb, tails=tails):
                st = tails[i]
                vps = ps_misc.tile([1, TB], F32, name=f"var_{tb}_{i}", tag="misc")
                _mm(nc, vps, onesc_sb[:, :], st["sq"][:, :], start=True, stop=True)
                # ones_col is 2.0 (for the sigmoid-via-tanh trick), so vps is
                # 2*sum(q^2); the extra 2 is folded into the sqrt scale.
                # The Sqrt lives in a different ACT table set than Exp; all 5
                # tail_vars of a block are emitted back-to-back so the table
                # reload happens twice per block instead of per chain.
                srow = rows.tile([1, TB], F32, name=f"srow_{tb}_{i}", tag="srow")
                nc.scalar.activation(srow[:], vps[:], AF.Sqrt,
                                     bias=eps_sb[0:1, 0:1], scale=0.5 / HD)
                vrow = rows.tile([1, TB], F32, name=f"vrow_{tb}_{i}", tag="row")
                nc.vector.reciprocal_approx_fast(out=vrow[:], in_=srow[:])
                vrep = wb.tile([128, TB], F32, name=f"vrep_{tb}_{i}", tag="vrep",
                               bufs=3)
                nc.gpsimd.partition_broadcast(vrep[:], vrow[0:1, :])
                st["vrep"] = vrep

            def tail_rot(i, tb=tb, tsl=tsl, tails=tails):
                # RoPE on the UN-normalized qsb: the per-token rstd is a
                # column scalar, so it commutes with the rotation and is
                # applied afterwards in tail_fm. This removes the PE rot
                # matmul from the sqrt->recip->bcast serial chain.
                st = tails[i]
                rot = ps_misc.tile([128, TB], F32, name=f"rot_{tb}_{i}", tag="misc")
                _mm(nc, rot[:], perm_sb[:, :], st["qsb"][:, :], start=True, stop=True)
                rot_sb = wb.tile([128, TB], MMDT, name=f"rsb_{tb}_{i}", tag="rsb")
                nc.scalar.copy(rot_sb[:], rot[:])
                t1 = wb.tile([128, TB], MMDT, name=f"t1_{tb}_{i}", tag="t1")
                nc.vector.tensor_mul(t1[:], st["qsb"][:], cos_sb[:, tsl])
                t2 = wb.tile([128, TB], MMDT, name=f"t2_{tb}_{i}", tag="t2")
                nc.vector.tensor_mul(t2[:], rot_sb[:], sin_sb[:, tsl])
                qtmp = wb.tile([128, TB], MMDT, name=f"qtmp_{tb}_{i}", tag="qn",
                               bufs=4)
                nc.vector.tensor_add(qtmp[:], t1[:], t2[:])
                st["qtmp"] = qtmp

            def tail_fm(i, tb=tb, tsl=tsl, tails=tails):
                st = tails[i]
                dst = qrope[:, st["h"], tsl] if st["kind"] == "q" else krope[:, tsl]
                nc.vector.tensor_mul(dst, st["qtmp"][:], st["vrep"][:])

            def accum_gate(h, tb=tb, xt=xt, tsl=tsl):
                ps = ps_acc.tile([128, TB], F32, name=f"psg_{tb}_{h}", tag="acc")
                for kt in range(NKT):
                    _mm(nc, ps, wg_sb[:, h, kt, :], xt[:, kt, :],
                        start=(kt == 0), stop=(kt == NKT - 1))
                # sigmoid(g) = 0.5*(1+tanh(g/2)); tanh shares the ACT table
                # set with exp so this causes no table reloads. The 0.5 is
                # folded into ones_col=2.0 (den becomes 2*den -> rden halves).
                nc.scalar.activation(sg[:, h, tsl], ps[:], AF.Tanh, scale=0.5)

            def accum_vT(tb=tb, xt=xt, vt_box=None):
                ps = ps_acc.tile([128, TB], F32, name=f"psvT_{tb}", tag="acc")
                for kt in range(NKT):
                    _mm(nc, ps, wv_sb[:, kt, :], xt[:, kt, :],
                        start=(kt == 0), stop=(kt == NKT - 1))
                vt = wb.tile([128, TB], MMDT, name=f"vt_{tb}", tag="vt")
                nc.vector.tensor_copy(vt[:], ps[:])
                vt_store[tb] = vt

            def transpose_v(tt, tb=tb):
                ti = tb * 4 + tt
                ps = ps_misc.tile([128, HD], MMDT, name=f"psvt_{tb}_{tt}", tag="misc")
                nc.tensor.transpose(ps[:], vt_store[tb][:, ts(tt, 128)], ident_sb[:])
                nc.vector.tensor_copy(v_sb[:, ti, :], ps[:])

            blocks = ([lambda i=i: accum_qk(i) for i in range(5)]
                      + [accum_vT])
            if not defer_gates:
                blocks += [lambda h=h: accum_gate(h) for h in range(NQ)]
            # All 5 tail_vars are emitted in one step so their Sqrt ops are
            # consecutive on the ACT queue (2 table reloads per block, not
            # 10). The rot matmuls move to later steps so the gate chains
            # provide PE cover while the serial sqrt->recip->bcast->qn chain
            # completes (otherwise the PE stalls ~2.5us once per block).
            tail_sched = {}
            for i in range(4):
                tail_sched.setdefault(i + 2, []).append(lambda i=i: tail_rot(i))
            order = [('v', 0), ('v', 1), ('f', 0), ('r', 4), ('v', 2),
                     ('f', 1), ('v', 3), ('f', 2), ('v', 4), ('f', 3),
                     ('f', 4)]
            fns = {'v': tail_var, 'f': tail_fm, 'r': tail_rot}
            tail_sched[6] = [(lambda k=k, i=i: fns[k](i)) for k, i in order]
            for tt in range(4):
                tail_sched.setdefault(7 + tt, []).append(lambda tt=tt: transpose_v(tt))

            def step(bi, blk):
                if blk is not None:
                    blk()
                for fn in tail_sched.get(bi + 1, ()):
                    fn()
            n_steps = max(len(blocks), max(tail_sched) if tail_sched else 0)
            steps = [lambda bi=bi: step(bi, blocks[bi] if bi < len(blocks) else None)
                     for bi in range(n_steps)]
            n = len(steps)
            groups = [steps[(n * h) // NQ:(n * (h + 1)) // NQ] for h in range(NQ)]
            if defer_gates:
                return groups, [lambda h=h: accum_gate(h) for h in range(NQ)]
            return groups

         def emit_phaseC(tb, fill_steps, pre_steps=None):
            """Attention for query block tb; after each head also emit the
            deferred o_proj group of tb-1 plus a slice of the next block's
            projection steps (PE work independent of the exp pipeline)."""
            tsl = ds(tb * TB, TB)
            nj = 4 * (tb + 1)
            for h in range(NQ):
                if pre_steps is not None and pre_steps[h] is not None:
                    pre_steps[h]()
                attn_ps = ps_acc.tile([128, TB], F32, name=f"attn_{tb}_{h}", tag="acc")
                den_ps = ps_den.tile([1, TB], F32, name=f"den_{tb}_{h}", tag="den")
                probs_t = [None] * nj

                def emit_scores(j, h=h, tb=tb, tsl=tsl, probs_t=probs_t):
                    o = j - 4 * tb
                    c0 = max(0, o) * 128          # first valid column in this tile
                    w = TB - c0
                    sp = ps_sc.tile([128, TB], F32, name=f"sc_{tb}_{h}_{j}", tag="sc")
                    _mm(nc, sp[:, c0:], krope[:, ts(j, 128)],
                        qrope[:, h, ds(tb * TB + c0, w)], start=True, stop=True)
                    pr = prp.tile([128, TB], MMDT, name=f"pr_{tb}_{h}_{j}", tag="pr")
                    nc.scalar.activation(pr[:, c0:], sp[:, c0:], AF.Exp, scale=SCALE)
                    if o >= 0:
                        nc.vector.tensor_mul(pr[:, c0:c0 + 128], pr[:, c0:c0 + 128],
                                             tri_sb[:, :])
                    probs_t[j] = pr

                # den: off-diagonal prob tiles are summed in groups of 4 on
                # DVE, one ones^T@sum matmul per group (saves 3 PE matmuls
                # per group); diagonal tiles keep per-j matmuls (partial
                # width, columns below c0 are unwritten).
                den_first = 3 if tb > 0 else 0

                def emit_av(j, h=h, tb=tb, nj=nj, attn_ps=attn_ps, den_ps=den_ps,
                            probs_t=probs_t, den_first=den_first):
                    o = j - 4 * tb
                    c0 = max(0, o) * 128
                    pr = probs_t[j]
                    _mm(nc, attn_ps[:, c0:], v_sb[:, j, :], pr[:, c0:],
                        start=(j == 0), stop=(j == nj - 1))
                    if o >= 0:
                        _mm(nc, den_ps[:, c0:], onesc_sb[:, :], pr[:, c0:],
                            start=(j == den_first), stop=(j == nj - 1))
                    elif j % 4 == 3:
                        p0, p1, p2, p3 = (probs_t[j - 3], probs_t[j - 2],
                                          probs_t[j - 1], probs_t[j])
                        s4 = wc.tile([128, TB], MMDT, name=f"s4_{tb}_{h}_{j}",
                                     tag="s4")
                        nc.vector.tensor_add(s4[:], p0[:], p1[:])
                        nc.vector.tensor_add(s4[:], s4[:], p2[:])
                        nc.vector.tensor_add(s4[:], s4[:], p3[:])
                        _mm(nc, den_ps[:, :], onesc_sb[:, :], s4[:, :],
                            start=(j == den_first), stop=False)

                # o_proj work of the previous query block is interleaved
                # into the j-loop at single-matmul granularity: the exp
                # pipeline produces one prob tile per ~600ns while PE only
                # has ~426ns of attn work per tile, so one extra independent
                # matmul per iteration keeps the PE from stalling on ACT.
                fillers = ([lambda idx=idx, hh=hh: emit_oproj_mm(tb - 1, idx, hh)
                            for idx in range(4 * h, 4 * h + 4)
                            for hh in range(NQ)]
                           if tb > 0 else [])
                fi = 0
                LOOK = 3
                for j in range(nj):
                    emit_scores(j)
                    if j >= LOOK:
                        emit_av(j - LOOK)
                    if j >= 2 and fi < len(fillers):
                        fillers[fi]()
                        fi += 1
                for j in range(max(0, nj - LOOK), nj):
                    emit_av(j)
                while fi < len(fillers):
                    fillers[fi]()
                    fi += 1

                # normalize + gate: attnT = attn/(2 den) * (1 + tanh(g/2))
                # (den here is 2*true_den because ones_col=2.0, so the 0.5 of
                # the sigmoid-from-tanh identity is already folded in)
                drow = rows.tile([1, TB], F32, name=f"drow_{tb}_{h}", tag="row")
                nc.vector.reciprocal_approx_fast(out=drow[:], in_=den_ps[:])
                rden = wc.tile([128, TB], F32, name=f"rden_{tb}_{h}", tag="rden")
                nc.gpsimd.partition_broadcast(rden[:], drow[0:1, :])
                g1 = wc.tile([128, TB], MMDT, name=f"g1_{tb}_{h}", tag="g1")
                nc.vector.tensor_mul(g1[:], attn_ps[:], rden[:])
                nc.vector.scalar_tensor_tensor(
                    out=attnT_t[tb][:, h, :], in0=sg[:, h, tsl], scalar=1.0,
                    in1=g1[:], op0=mybir.AluOpType.add, op1=mybir.AluOpType.mult)

                if fill_steps:
                    for s in fill_steps[h]:
                        s()

         # driver: B0 (gates deferred into C0), then C(tb) with B(tb+1)
         # groups interleaved
         b0_groups, b0_gates = make_phaseB(0, defer_gates=True)
         b0_steps = [st for grp in b0_groups for st in grp]
         for st in b0_steps[:6]:      # chains + vT + var batch
            st()
         b0_gates[0]()                # gate chain h0: PE cover for the
         for st in b0_steps[6:]:      # sqrt->qn chain feeding the rots
            st()
         nextB = make_phaseB(1)
         c0_pre = [None] + b0_gates[1:]
         for tb in range(NTB):
            emit_phaseC(tb, nextB, pre_steps=c0_pre if tb == 0 else None)
            nextB = make_phaseB(tb + 2) if tb + 2 < NTB else None

         # final o_proj for the last query block
         for g in range(4):
            emit_oproj_group(NTB - 1, g)


_CACHED = {}
_REPS = [1]


def _build(reps=None):
    if reps is None:
        reps = int(os.environ.get("KERNEL_REPS", "1"))
    if reps in _CACHED:
        return _CACHED[reps]
    _REPS[0] = reps
    nc = bacc.Bacc("TRN2", target_bir_lowering=False, debug=False, num_devices=8)
    io = {}
    def din(name, shape, dt):
        io[name] = nc.dram_tensor(name, shape, dt, kind="ExternalInput").ap()
    din("xb", [NTB, 128, NKT, TB], MMDT)
    din("wq", [NQ, 128, NKT, HD], MMDT)
    din("wg", [NQ, 128, NKT, HD], MMDT)
    din("wk", [128, NKT, HD], MMDT)
    din("wv", [128, NKT, HD], MMDT)
    din("wo", [128, NQ, D], MMDT)
    din("cosT", [HD, T], MMDT)
    din("sinT", [HD, T], MMDT)
    din("perm", [HD, HD], MMDT)
    din("qw_col", [HD, 1], F32)
    din("kw_col", [HD, 1], F32)
    din("tri", [128, 128], MMDT)
    din("ones_col", [128, 1], MMDT)
    io["out2"] = nc.dram_tensor("out2", [NTT, 2, 128, 2 * TB], MMDT,
                                kind="ExternalOutput").ap()

    with tile.TileContext(nc, num_cores=8) as tc:
        _emit(tc, io)
    nc.compile()
    _CACHED[reps] = nc
    return nc


def _prep_in_maps(inputs):
    hidden = np.asarray(inputs["hidden_BTD"], np.float32)
    cos = np.asarray(inputs["cos_BTK"], np.float32)
    sin = np.asarray(inputs["sin_BTK"], np.float32)
    w_q = np.asarray(inputs["w_q"], np.float32)
    w_k = np.asarray(inputs["w_k"], np.float32)
    w_v = np.asarray(inputs["w_v"], np.float32)
    w_o = np.asarray(inputs["w_o"], np.float32)
    qw = np.asarray(inputs["q_norm_w"], np.float32)
    kw = np.asarray(inputs["k_norm_w"], np.float32)

    wq4 = w_q.reshape(D, NH, 2 * HD)

    def cvt(x):
        return np.ascontiguousarray(np.asarray(x, np.float32).astype(NPMM))

    # upper-tri-inclusive [128,128]: tri[jl, cc] = 1 iff jl <= cc
    tri = np.triu(np.ones((128, 128), np.float32))

    perm = np.zeros((128, 128), np.float32)
    perm[np.arange(64), np.arange(64) + 64] = 1.0
    perm[np.arange(64, 128), np.arange(64, 128) - 64] = -1.0

    def blk_w(w):  # [D, C] -> [128, NKT, C] with d = kt*128 + p
        return w.reshape(NKT, 128, -1).transpose(1, 0, 2)

    in_maps = []
    for c in range(8):
        b, g = divmod(c, 4)
        heads = list(range(4 * g, 4 * g + 4))
        xT = hidden[b].T                                   # [D, T]
        # [tb][p][kt][t'] blocks, contiguous per block
        xb = xT.reshape(NKT, 128, NTB, TB).transpose(2, 1, 0, 3)
        wq_h = np.stack([blk_w(wq4[:, h, :HD]) for h in heads])   # [NQ,128,NKT,HD]
        wg_h = np.stack([blk_w(wq4[:, h, HD:]) for h in heads])
        wo_g = w_o[4 * g * HD:(4 * g + 4) * HD, :]         # [DH, D]
        m = {
            "xb": cvt(xb),
            "wq": cvt(wq_h),
            "wg": cvt(wg_h),
            "wk": cvt(blk_w(w_k[:, g * HD:(g + 1) * HD])),
            "wv": cvt(blk_w(w_v[:, g * HD:(g + 1) * HD])),
            "wo": cvt(wo_g.reshape(NQ, 128, D).transpose(1, 0, 2)),
            "cosT": cvt(cos[b].T),
            "sinT": cvt(sin[b].T),
            "perm": cvt(perm),
            "qw_col": np.ascontiguousarray(qw[:, None]),
            "kw_col": np.ascontiguousarray(kw[:, None]),
            "tri": cvt(tri),
            # 2.0 (not 1.0): folds the 0.5 of sigmoid(x)=0.5*(1+tanh(x/2))
            # into the softmax denominator; the var matmul's extra 2 is
            # compensated in the Sqrt scale.
            "ones_col": cvt(np.full((128, 1), 2.0, np.float32)),
        }
        in_maps.append(m)
    return in_maps


def run(inputs, **spmd_kwargs):
    """Build+run; returns (full_output [B,T,D] fp32, BassKernelResults)."""
    nc = _build()
    in_maps = _prep_in_maps(inputs)
    res = run_bass_kernel_spmd(nc, in_maps, core_ids=list(range(8)), **spmd_kwargs)
    out = np.zeros((B, T, D), np.float32)
    for c in range(8):
        o = np.asarray(res.results[c]["out2"], np.float32)  # [NTT,2,128,1024]
        out[c // 4] += o.transpose(0, 2, 1, 3).reshape(T, D)
    return out, res


def kernel(**inputs):
    out, _ = run(inputs)
    return out
